# revision 2
# baseline (speedup 1.0000x reference)
"""Distributed masked-attention kernel for 8 TRN2 NeuronCores.

Reference computation (B=2, L=1024, D=1024, H=16, DH=64):
    Qz, Kz = masked Q, K;  Qp/Kp/Vp = projections (V = K)
    per-head attention with outer-product validity mask, softmax scaled
    by 1/sqrt(D);  O = Qp + attn;  out = O + relu(mask_q(O @ Wo.T))

Sharding: core c = 4*b + g handles batch b, head group g (4 heads,
feature block e = [256g, 256g+256)).  All activations are kept
feature-major ("X.T" = [features, tokens]) so the TensorEngine can
contract along partitions without any transposes; the host pre-transposes
Q, K and the weight shards.

Softmax details: scores are small (|S/32| < ~2) so no max-subtraction is
needed; exp(S/32 + bias_k) with bias_k = -30000 at masked k gives exact
zeros; the denominator comes from a ones-column appended to V (M=65
matmul) plus a 1e30 PSUM prefill at masked-q columns so 1/denom ~ 0 there.

A single 8-core AllToAll (128KB chunks) converts the head-sharded O.T
into token-sharded slices (128 tokens of each batch per core) for the
output projection epilogue; the host reassembles the final [2,1024,1024].
"""
import numpy as np

B, L, D = 2, 1024, 1024
H, DH = 16, 64
NCORES = 8
HPC = 4          # heads per core
EB = 256         # feature block per core
NEG = -30000.0   # masked-k bias (exp -> exact 0)
BIG = 1e30       # masked-q denominator prefill

TRACE = False
TRACE_KWARGS = {}
LAST_RESULTS = None

_compiled = None


def _build():
    import concourse.bacc as bacc
    import concourse.tile as tile
    from concourse import mybir

    f32 = mybir.dt.float32
    f32r = mybir.dt.float32r
    bf16 = mybir.dt.bfloat16
    EXP = mybir.ActivationFunctionType.Exp
    RELU = mybir.ActivationFunctionType.Relu
    COPY = mybir.ActivationFunctionType.Copy

    nc = bacc.Bacc("TRN2", target_bir_lowering=False, debug=False,
                   num_devices=NCORES)

    qt = nc.dram_tensor("qt", [D, L], f32r, kind="ExternalInput")
    kt = nc.dram_tensor("kt", [D, L], f32r, kind="ExternalInput")
    wq = nc.dram_tensor("wq", [D, EB], f32r, kind="ExternalInput")
    wk = nc.dram_tensor("wk", [D, EB], f32r, kind="ExternalInput")
    wv = nc.dram_tensor("wv", [D, EB], f32r, kind="ExternalInput")
    wo = nc.dram_tensor("wo", [D, D], f32r, kind="ExternalInput")
    bk = nc.dram_tensor("bk", [128, 8], f32, kind="ExternalInput")
    e64 = nc.dram_tensor("e64", [1, 65], f32r, kind="ExternalInput")
    mvn = nc.dram_tensor("mvn", [1, L], f32r, kind="ExternalInput")
    mv = nc.dram_tensor("mv", [1, L], f32, kind="ExternalInput")
    out = nc.dram_tensor("out", [D, 256], f32, kind="ExternalOutput")

    with tile.TileContext(nc) as tc:
        with (
            tc.tile_pool(name="sb", bufs=1) as sb,
            tc.tile_pool(name="rot", bufs=3) as rot,
            tc.tile_pool(name="ps_big", bufs=3, space="PSUM") as ps_big,
            tc.tile_pool(name="ps_at", bufs=3, space="PSUM") as ps_at,
            tc.tile_pool(name="ps_sm", bufs=2, space="PSUM") as ps_sm,
            tc.tile_pool(name="dram", bufs=1, space="DRAM") as dram,
        ):
            # ---- constants / masks ----
            bk_t = sb.tile([128, 8], f32, tag="bk")
            e64_t = sb.tile([1, 65], f32r, tag="e64")
            mvn_t = sb.tile([1, L], f32r, tag="mvn")
            mv_t = sb.tile([1, L], f32, tag="mv")
            nc.sync.dma_start(bk_t[:], bk[:])
            nc.sync.dma_start(e64_t[:], e64[:])
            nc.sync.dma_start(mvn_t[:], mvn[:])
            nc.sync.dma_start(mv_t[:], mv[:])
            mvbc = sb.tile([128, L], f32, tag="mvbc")
            nc.gpsimd.partition_broadcast(mvbc[:], mv_t[:])

            # ---- input DMAs ----
            qt_t = [sb.tile([128, L], f32r, tag=f"qt{i}", name=f"qt{i}") for i in range(8)]
            kt_t = [sb.tile([128, L], f32r, tag=f"kt{i}", name=f"kt{i}") for i in range(8)]
            wq_t = [sb.tile([128, EB], f32r, tag=f"wq{i}", name=f"wq{i}") for i in range(8)]
            wk_t = [sb.tile([128, EB], f32r, tag=f"wk{i}", name=f"wk{i}") for i in range(8)]
            wv_t = [sb.tile([128, EB], f32r, tag=f"wv{i}", name=f"wv{i}") for i in range(8)]
            for i in range(8):
                nc.sync.dma_start(qt_t[i][:], qt[128 * i:128 * (i + 1), :])
                nc.sync.dma_start(wq_t[i][:], wq[128 * i:128 * (i + 1), :])
            for i in range(8):
                nc.sync.dma_start(kt_t[i][:], kt[128 * i:128 * (i + 1), :])
                nc.sync.dma_start(wk_t[i][:], wk[128 * i:128 * (i + 1), :])
                nc.sync.dma_start(wv_t[i][:], wv[128 * i:128 * (i + 1), :])

            # ---- phase 1: projections (feature-major Qp.T/Kp.T, natural Vp) ----
            qpt = [sb.tile([128, L], f32r, tag=f"qpt{i}", name=f"qpt{i}") for i in range(2)]
            kpt = [sb.tile([128, L], f32r, tag=f"kpt{i}", name=f"kpt{i}") for i in range(2)]
            for et in range(2):
                for qc in range(2):
                    pq = ps_big.tile([128, 512], f32, tag="big")
                    for dc in range(8):
                        nc.tensor.matmul(
                            pq[:], wq_t[dc][:, 128 * et:128 * (et + 1)],
                            qt_t[dc][:, 512 * qc:512 * (qc + 1)],
                            start=(dc == 0), stop=(dc == 7))
                    # evict with query-mask fold-in (O residual uses masked Qp)
                    nc.vector.tensor_mul(
                        qpt[et][:, 512 * qc:512 * (qc + 1)], pq[:],
                        mvbc[:, 512 * qc:512 * (qc + 1)])
            for et in range(2):
                for qc in range(2):
                    pk = ps_big.tile([128, 512], f32, tag="big")
                    for dc in range(8):
                        nc.tensor.matmul(
                            pk[:], wk_t[dc][:, 128 * et:128 * (et + 1)],
                            kt_t[dc][:, 512 * qc:512 * (qc + 1)],
                            start=(dc == 0), stop=(dc == 7))
                    nc.scalar.activation(
                        kpt[et][:, 512 * qc:512 * (qc + 1)], pk[:], COPY)
            # Vp natural [k-tokens, e] with ones column per head (65-stride)
            vpa = [sb.tile([128, 65 * HPC], bf16, tag=f"vpa{i}", name=f"vpa{i}") for i in range(8)]
            for tt in range(8):
                nc.gpsimd.memset(vpa[tt][:], 1.0)
                pv = ps_sm.tile([128, EB], f32, tag="sm")
                for dc in range(8):
                    nc.tensor.matmul(
                        pv[:], kt_t[dc][:, 128 * tt:128 * (tt + 1)], wv_t[dc][:],
                        start=(dc == 0), stop=(dc == 7))
                for h in range(HPC):
                    nc.vector.tensor_copy(
                        vpa[tt][:, 65 * h:65 * h + 64],
                        pv[:, 64 * h:64 * (h + 1)])

            # ---- epilogue weights (prefetch; ordered after phase-1 loads) ----
            wo_t = [sb.tile([128, D], f32r, tag=f"wo{i}", name=f"wo{i}") for i in range(8)]
            for i in range(8):
                nc.sync.dma_start(wo_t[i][:], wo[128 * i:128 * (i + 1), :])

            # ---- phase 2: attention (head pairs for PE row-packing) ----
            attn = [sb.tile([128, L], f32, tag=f"attn{i}", name=f"attn{i}") for i in range(2)]
            for qc in range(2):
                qs = slice(512 * qc, 512 * (qc + 1))
                for hp in (0, 2):
                    et = hp // 2
                    ats = []
                    for h in (hp, hp + 1):
                        at = ps_at.tile([65, 512], f32, tag="at")
                        nc.tensor.matmul(at[:], e64_t[:], mvn_t[:, qs],
                                         start=True, stop=False)
                        ats.append(at)
                    for ki in range(8):
                        ks = slice(128 * ki, 128 * (ki + 1))
                        ps_pair = []
                        for h in (hp, hp + 1):
                            ro = 64 * (h % 2)
                            s_ps = ps_big.tile([128, 512], f32, tag="big")
                            nc.tensor.matmul(
                                s_ps[:], kpt[et][ro:ro + 64, ks],
                                qpt[et][ro:ro + 64, qs],
                                start=True, stop=True)
                            ps_pair.append(s_ps)
                        for j, h in enumerate((hp, hp + 1)):
                            p_t = rot.tile([128, 512], bf16, tag="p")
                            nc.scalar.activation(
                                p_t[:], ps_pair[j][:], EXP,
                                bias=bk_t[:, ki:ki + 1], scale=1.0 / 32)
                            nc.tensor.matmul(
                                ats[j][:], vpa[ki][:, 65 * h:65 * h + 65],
                                p_t[:], start=False, stop=(ki == 7))
                    for j, h in enumerate((hp, hp + 1)):
                        ro = 64 * (h % 2)
                        den = rot.tile([1, 512], f32, tag="den")
                        nc.vector.tensor_copy(den[:], ats[j][64:65, :])
                        rcp = rot.tile([1, 512], f32, tag="rcp")
                        nc.vector.reciprocal_approx_fast(rcp[:], den[:])
                        bc = rot.tile([64, 512], f32, tag="bc")
                        nc.gpsimd.partition_broadcast(bc[:], rcp[:])
                        nc.vector.tensor_mul(
                            attn[et][ro:ro + 64, qs], ats[j][0:64, :], bc[:])

            # ---- phase 3: residual, AllToAll redistribute ----
            inb = dram.tile([2048, 128], f32r)
            outb = dram.tile([2048, 128], f32r)
            ot = [sb.tile([128, L], f32r, tag=f"ot{i}", name=f"ot{i}") for i in range(2)]
            from concourse import mybir as _mb
            for et in range(2):
                nc.vector.tensor_add(ot[et][:], qpt[et][:].bitcast(f32),
                                     attn[et][:])
                for j in range(8):
                    nc.sync.dma_start(
                        inb[256 * j + 128 * et:256 * j + 128 * (et + 1), :],
                        ot[et][:, 128 * j:128 * (j + 1)])
            nc.gpsimd.collective_compute(
                "AllToAll", _mb.AluOpType.bypass,
                replica_groups=[list(range(NCORES))],
                ins=[inb[:].opt()], outs=[outb[:].opt()])
            ot_sl = [sb.tile([128, 256], f32r, tag=f"osl{i}", name=f"osl{i}") for i in range(8)]
            for dt_ in range(8):
                r0 = 256 * (dt_ // 2) + 128 * (dt_ % 2)
                r1 = 256 * (4 + dt_ // 2) + 128 * (dt_ % 2)
                nc.sync.dma_start(ot_sl[dt_][:, 0:128], outb[r0:r0 + 128, :])
                nc.sync.dma_start(ot_sl[dt_][:, 128:256], outb[r1:r1 + 128, :])

            # ---- phase 4: output projection epilogue ----
            for et in range(8):
                fp = ps_sm.tile([128, 256], f32, tag="sm")
                for dc in range(8):
                    nc.tensor.matmul(
                        fp[:], wo_t[dc][:, 128 * et:128 * (et + 1)],
                        ot_sl[dc][:], start=(dc == 0), stop=(dc == 7))
                ff = rot.tile([128, 256], f32, tag="ff")
                nc.scalar.activation(ff[:], fp[:], RELU)
                o_t = rot.tile([128, 256], f32, tag="outt")
                nc.vector.tensor_add(o_t[:], ff[:], ot_sl[et][:].bitcast(f32))
                nc.sync.dma_start(out[128 * et:128 * (et + 1), :], o_t[:])

    nc.compile()
    return nc


def _get_compiled():
    global _compiled
    if _compiled is None:
        _compiled = _build()
    return _compiled


def kernel(Q, K, mask_Q, mask_K, Wq, Wk, Wv, Wo):
    global LAST_RESULTS
    from concourse.bass_utils import run_bass_kernel_spmd

    Q = np.asarray(Q, np.float32)
    K = np.asarray(K, np.float32)
    mask_Q = np.asarray(mask_Q, bool)
    mask_K = np.asarray(mask_K, bool)
    Wq = np.asarray(Wq, np.float32)
    Wk = np.asarray(Wk, np.float32)
    Wv = np.asarray(Wv, np.float32)
    Wo = np.asarray(Wo, np.float32)

    nc = _get_compiled()

    e64v = np.zeros((1, 65), np.float32)
    e64v[0, 64] = BIG
    wot = np.ascontiguousarray(Wo.T)
    in_maps = []
    for c in range(NCORES):
        b, g = c // 4, c % 4
        eb = slice(EB * g, EB * (g + 1))
        bias = np.where(mask_K[b], NEG, 0.0).astype(np.float32)
        in_maps.append({
            "qt": np.ascontiguousarray(Q[b].T),
            "kt": np.ascontiguousarray(K[b].T),
            "wq": np.ascontiguousarray(Wq[eb, :].T),
            "wk": np.ascontiguousarray(Wk[eb, :].T),
            "wv": np.ascontiguousarray(Wv[eb, :].T),
            "wo": wot,
            "bk": np.ascontiguousarray(bias.reshape(8, 128).T),
            "e64": e64v,
            "mvn": mask_Q[b].astype(np.float32)[None, :],
            "mv": (~mask_Q[b]).astype(np.float32)[None, :],
        })

    res = run_bass_kernel_spmd(nc, in_maps, core_ids=list(range(NCORES)),
                               trace=TRACE, **TRACE_KWARGS)
    LAST_RESULTS = res

    full = np.empty((B, L, D), np.float32)
    for c in range(NCORES):
        o = res.results[c]["out"]          # [1024 e, 256] (128 tok per batch)
        full[0, 128 * c:128 * (c + 1), :] = o[:, 0:128].T
        full[1, 128 * c:128 * (c + 1), :] = o[:, 128:256].T
    return full


# revision 4
# speedup vs baseline: 1.2695x; 1.2695x over previous
"""Distributed masked-attention kernel for 8 TRN2 NeuronCores (v2).

Reference computation (B=2, L=1024, D=1024, H=16, DH=64):
    Qz, Kz = masked Q, K;  Qp/Kp/Vp = projections (V = K)
    per-head attention with outer-product validity mask, softmax scaled
    by 1/sqrt(D);  O = Qp + attn;  out = O + relu(mask_q(O @ Wo.T))

Sharding: core c = 4*b + g handles batch b, head group g (4 heads,
feature block e = [256g, 256g+256)).  Activations are feature-major
("X.T" = [features, tokens]) so the TensorEngine contracts along
partitions without transposes; the host pre-transposes Q, K and the
weight shards and casts everything to bf16 (PSUM accumulation is f32).

Softmax: scores are small (|S/32| < ~2) so no max-subtraction; exp(S/32 +
bias_k) with bias_k = -30000 at masked k gives exact zeros; the
denominator comes from a ones-column appended to V (M=65 matmul) plus a
1e30 PSUM prefill at masked-q columns so 1/denom ~ 0 there.

Two AllToAll collectives (one per 2-head feature half, 0.5MB each)
convert head-sharded O.T into token-sharded slices (128 tokens of each
batch per core); the first one is hidden under the second half of
attention.  The host reassembles the final [2,1024,1024].
"""
import numpy as np

B, L, D = 2, 1024, 1024
H, DH = 16, 64
NCORES = 8
HPC = 4          # heads per core
EB = 256         # feature block per core
NEG = -30000.0   # masked-k bias (exp -> exact 0)
BIG = 1e30       # masked-q denominator prefill

TRACE = False
TRACE_KWARGS = {}
LAST_RESULTS = None

_compiled = None


def _build():
    import concourse.bacc as bacc
    import concourse.tile as tile
    from concourse import mybir

    f32 = mybir.dt.float32
    bf16 = mybir.dt.bfloat16
    EXP = mybir.ActivationFunctionType.Exp
    RELU = mybir.ActivationFunctionType.Relu

    nc = bacc.Bacc("TRN2", target_bir_lowering=False, debug=False,
                   num_devices=NCORES)

    qt = nc.dram_tensor("qt", [D, L], bf16, kind="ExternalInput")
    kt = nc.dram_tensor("kt", [D, L], bf16, kind="ExternalInput")
    wq = nc.dram_tensor("wq", [D, EB], bf16, kind="ExternalInput")
    wk = nc.dram_tensor("wk", [D, EB], bf16, kind="ExternalInput")
    wv = nc.dram_tensor("wv", [D, EB], bf16, kind="ExternalInput")
    wo = nc.dram_tensor("wo", [D, D], bf16, kind="ExternalInput")
    bk = nc.dram_tensor("bk", [128, 8], f32, kind="ExternalInput")
    e64 = nc.dram_tensor("e64", [1, 65], bf16, kind="ExternalInput")
    mvn = nc.dram_tensor("mvn", [1, L], bf16, kind="ExternalInput")
    mv = nc.dram_tensor("mv", [1, L], f32, kind="ExternalInput")
    out = nc.dram_tensor("out", [D, 256], f32, kind="ExternalOutput")

    with tile.TileContext(nc) as tc:
        with (
            tc.tile_pool(name="sb", bufs=1) as sb,
            tc.tile_pool(name="rot", bufs=3) as rot,
            tc.tile_pool(name="ps_big", bufs=2, space="PSUM") as ps_big,
            tc.tile_pool(name="ps_at", bufs=3, space="PSUM") as ps_at,
            tc.tile_pool(name="ps_sm", bufs=1, space="PSUM") as ps_sm,
            tc.tile_pool(name="dram", bufs=1, space="DRAM") as dram,
        ):
            # ---- constants / masks (tiny, first) ----
            bk_t = sb.tile([128, 8], f32, tag="bk")
            e64_t = sb.tile([1, 65], bf16, tag="e64")
            mvn_t = sb.tile([1, L], bf16, tag="mvn")
            mv_t = sb.tile([1, L], f32, tag="mv")
            nc.sync.dma_start(bk_t[:], bk[:])
            nc.sync.dma_start(e64_t[:], e64[:])
            nc.sync.dma_start(mvn_t[:], mvn[:])
            nc.sync.dma_start(mv_t[:], mv[:])
            mvbc = sb.tile([128, L], f32, tag="mvbc")
            nc.gpsimd.partition_broadcast(mvbc[:], mv_t[:])

            # ---- input DMAs, interleaved so dc-major compute starts early ----
            qt_t = [sb.tile([128, L], bf16, tag=f"qt{i}", name=f"qt{i}")
                    for i in range(8)]
            kt_t = [sb.tile([128, L], bf16, tag=f"kt{i}", name=f"kt{i}")
                    for i in range(8)]
            wq_t = [sb.tile([128, EB], bf16, tag=f"wq{i}", name=f"wq{i}")
                    for i in range(8)]
            wk_t = [sb.tile([128, EB], bf16, tag=f"wk{i}", name=f"wk{i}")
                    for i in range(8)]
            wv_t = [sb.tile([128, EB], bf16, tag=f"wv{i}", name=f"wv{i}")
                    for i in range(8)]
            for i in range(8):
                nc.sync.dma_start(wq_t[i][:], wq[128 * i:128 * (i + 1), :])
                nc.sync.dma_start(qt_t[i][:], qt[128 * i:128 * (i + 1), :])
            for i in range(8):
                nc.sync.dma_start(wk_t[i][:], wk[128 * i:128 * (i + 1), :])
                nc.sync.dma_start(kt_t[i][:], kt[128 * i:128 * (i + 1), :])
                nc.sync.dma_start(wv_t[i][:], wv[128 * i:128 * (i + 1), :])

            # ---- HAM warmup: a few dummy matmuls while DMAs land ----
            warm_ps = ps_at.tile([65, 512], f32, tag="at", name="warm_ps")
            for w in range(8):
                nc.tensor.matmul(warm_ps[:], e64_t[:],
                                 mvn_t[:, 0:512], start=(w == 0), stop=(w == 7))

            # ---- phase 1: projections, dc-major so PE starts on first chunk --
            qpt = [sb.tile([128, L], bf16, tag=f"qpt{i}", name=f"qpt{i}")
                   for i in range(2)]
            kpt = [sb.tile([128, L], bf16, tag=f"kpt{i}", name=f"kpt{i}")
                   for i in range(2)]
            for dst, w_t, x_t in ((qpt, wq_t, qt_t), (kpt, wk_t, kt_t)):
                pj = [ps_big.tile([128, L], f32, tag="big", name=f"pj{id(dst)}{et}")
                      for et in range(2)]
                for dc in range(8):
                    for et in range(2):
                        for qc in range(2):
                            nc.tensor.matmul(
                                pj[et][:, 512 * qc:512 * (qc + 1)],
                                w_t[dc][:, 128 * et:128 * (et + 1)],
                                x_t[dc][:, 512 * qc:512 * (qc + 1)],
                                start=(dc == 0), stop=(dc == 7))
                for et in range(2):
                    if dst is qpt:
                        # fold query-mask into Qp (residual uses masked Qp)
                        nc.vector.tensor_mul(dst[et][:], pj[et][:], mvbc[:])
                    else:
                        nc.vector.tensor_copy(dst[et][:], pj[et][:])

            # Vp natural [k-tokens, e] with ones column per head (65-stride)
            vpa = [sb.tile([128, 65 * HPC], bf16, tag=f"vpa{i}", name=f"vpa{i}")
                   for i in range(8)]
            for tt in range(8):
                nc.gpsimd.memset(vpa[tt][:], 1.0)
                pv = ps_sm.tile([128, EB], f32, tag="sm", name=f"pv{tt}")
                for dc in range(8):
                    nc.tensor.matmul(
                        pv[:], kt_t[dc][:, 128 * tt:128 * (tt + 1)], wv_t[dc][:],
                        start=(dc == 0), stop=(dc == 7))
                for h in range(HPC):
                    nc.vector.tensor_copy(
                        vpa[tt][:, 65 * h:65 * h + 64],
                        pv[:, 64 * h:64 * (h + 1)])

            # ---- epilogue weights (prefetch, after phase-1 loads) ----
            wo_t = [sb.tile([128, D], bf16, tag=f"wo{i}", name=f"wo{i}")
                    for i in range(8)]
            for i in range(8):
                nc.sync.dma_start(wo_t[i][:], wo[128 * i:128 * (i + 1), :])

            # ---- phase 2+3: attention per head pair; A2A per feature half ----
            # A2A chunk layout (per half): [8 chunks x 128 e-rows x 128 t] bf16.
            inb = [dram.tile([1024, 128], bf16, tag=f"inb{i}", name=f"inb{i}") for i in range(2)]
            outb = [dram.tile([1024, 128], bf16, tag=f"outb{i}", name=f"outb{i}") for i in range(2)]
            attn = [sb.tile([128, L], bf16, tag=f"attn{i}", name=f"attn{i}")
                    for i in range(2)]
            ot_sl = [sb.tile([128, 256], bf16, tag=f"osl{i}", name=f"osl{i}")
                     for i in range(8)]
            ot = [sb.tile([128, L], bf16, tag=f"ot{i}", name=f"ot{i}")
                  for i in range(2)]
            from concourse import mybir as _mb

            for hp in (0, 2):
                et = hp // 2
                for qc in range(2):
                    qs = slice(512 * qc, 512 * (qc + 1))
                    ats = []
                    for h in (hp, hp + 1):
                        at = ps_at.tile([65, 512], f32, tag="at",
                                        name=f"at{h}_{qc}")
                        nc.tensor.matmul(at[:], e64_t[:], mvn_t[:, qs],
                                         start=True, stop=False)
                        ats.append(at)
                    # software pipeline: S(k) issued ahead of attn(k-1)
                    p_prev = None
                    for ki in range(8):
                        ks = slice(128 * ki, 128 * (ki + 1))
                        s_ps = ps_big.tile([128, 1024], f32, tag="big",
                                           name=f"s{hp}_{qc}_{ki}")
                        for j in range(2):
                            ro = 64 * j
                            nc.tensor.matmul(
                                s_ps[:, 512 * j:512 * (j + 1)],
                                kpt[et][ro:ro + 64, ks],
                                qpt[et][ro:ro + 64, qs],
                                start=True, stop=True)
                        p_t = rot.tile([128, 1024], bf16, tag="p",
                                       name=f"p{hp}_{qc}_{ki}")
                        nc.scalar.activation(p_t[:], s_ps[:], EXP,
                                             bias=bk_t[:, ki:ki + 1],
                                             scale=1.0 / 32)
                        if p_prev is not None:
                            kp_, pp = p_prev
                            for j, h in enumerate((hp, hp + 1)):
                                nc.tensor.matmul(
                                    ats[j][:], vpa[kp_][:, 65 * h:65 * h + 65],
                                    pp[:, 512 * j:512 * (j + 1)],
                                    start=False, stop=False)
                        p_prev = (ki, p_t)
                    kp_, pp = p_prev
                    for j, h in enumerate((hp, hp + 1)):
                        nc.tensor.matmul(
                            ats[j][:], vpa[kp_][:, 65 * h:65 * h + 65],
                            pp[:, 512 * j:512 * (j + 1)],
                            start=False, stop=True)
                    # normalize: attn = at[0:64] / denom  (denom row 64)
                    for j, h in enumerate((hp, hp + 1)):
                        ro = 64 * (h % 2)
                        den = rot.tile([1, 512], f32, tag="den")
                        nc.vector.tensor_copy(den[:], ats[j][64:65, :])
                        rcp = rot.tile([1, 512], f32, tag="rcp")
                        nc.vector.reciprocal_approx_fast(rcp[:], den[:])
                        bc = rot.tile([64, 512], f32, tag="bc")
                        nc.gpsimd.partition_broadcast(bc[:], rcp[:])
                        nc.vector.tensor_mul(
                            attn[et][ro:ro + 64, qs], ats[j][0:64, :], bc[:])
                # residual + A2A half for this head pair
                nc.vector.tensor_add(ot[et][:], qpt[et][:], attn[et][:])
                for j in range(8):
                    nc.sync.dma_start(
                        inb[et][128 * j:128 * (j + 1), :],
                        ot[et][:, 128 * j:128 * (j + 1)])
                nc.gpsimd.collective_compute(
                    "AllToAll", _mb.AluOpType.bypass,
                    replica_groups=[list(range(NCORES))],
                    ins=[inb[et][:].opt()], outs=[outb[et][:].opt()])
                for blk in range(4):
                    dt_ = 2 * blk + et
                    nc.sync.dma_start(ot_sl[dt_][:, 0:128],
                                      outb[et][128 * blk:128 * (blk + 1), :])
                    nc.sync.dma_start(ot_sl[dt_][:, 128:256],
                                      outb[et][128 * (4 + blk):128 * (5 + blk), :])

            # ---- phase 4: output projection epilogue ----
            for et in range(8):
                fp = ps_sm.tile([128, 256], f32, tag="sm", name=f"fp{et}")
                for i, dc in enumerate((0, 2, 4, 6, 1, 3, 5, 7)):
                    nc.tensor.matmul(
                        fp[:], wo_t[dc][:, 128 * et:128 * (et + 1)],
                        ot_sl[dc][:], start=(i == 0), stop=(i == 7))
                ff = rot.tile([128, 256], f32, tag="ff")
                nc.scalar.activation(ff[:], fp[:], RELU)
                o_t = rot.tile([128, 256], f32, tag="outt")
                nc.vector.tensor_add(o_t[:], ff[:], ot_sl[et][:])
                nc.sync.dma_start(out[128 * et:128 * (et + 1), :], o_t[:])

    nc.compile()
    return nc


def _get_compiled():
    global _compiled
    if _compiled is None:
        _compiled = _build()
    return _compiled


def kernel(Q, K, mask_Q, mask_K, Wq, Wk, Wv, Wo):
    global LAST_RESULTS
    import ml_dtypes
    from concourse.bass_utils import run_bass_kernel_spmd

    bf = ml_dtypes.bfloat16
    Q = np.asarray(Q, np.float32)
    K = np.asarray(K, np.float32)
    mask_Q = np.asarray(mask_Q, bool)
    mask_K = np.asarray(mask_K, bool)
    Wq = np.asarray(Wq, np.float32)
    Wk = np.asarray(Wk, np.float32)
    Wv = np.asarray(Wv, np.float32)
    Wo = np.asarray(Wo, np.float32)

    nc = _get_compiled()

    e64v = np.zeros((1, 65), np.float32)
    e64v[0, 64] = BIG
    wot = np.ascontiguousarray(Wo.T.astype(bf))
    in_maps = []
    for c in range(NCORES):
        b, g = c // 4, c % 4
        eb = slice(EB * g, EB * (g + 1))
        bias = np.where(mask_K[b], NEG, 0.0).astype(np.float32)
        in_maps.append({
            "qt": np.ascontiguousarray(Q[b].T.astype(bf)),
            "kt": np.ascontiguousarray(K[b].T.astype(bf)),
            "wq": np.ascontiguousarray(Wq[eb, :].T.astype(bf)),
            "wk": np.ascontiguousarray(Wk[eb, :].T.astype(bf)),
            "wv": np.ascontiguousarray(Wv[eb, :].T.astype(bf)),
            "wo": wot,
            "bk": np.ascontiguousarray(bias.reshape(8, 128).T),
            "e64": e64v.astype(bf),
            "mvn": mask_Q[b].astype(bf)[None, :],
            "mv": (~mask_Q[b]).astype(np.float32)[None, :],
        })

    res = run_bass_kernel_spmd(nc, in_maps, core_ids=list(range(NCORES)),
                               trace=TRACE, **TRACE_KWARGS)
    LAST_RESULTS = res

    full = np.empty((B, L, D), np.float32)
    for c in range(NCORES):
        o = res.results[c]["out"]          # [1024 e, 256] (128 tok per batch)
        full[0, 128 * c:128 * (c + 1), :] = o[:, 0:128].T
        full[1, 128 * c:128 * (c + 1), :] = o[:, 128:256].T
    return full


# revision 6
# speedup vs baseline: 1.3116x; 1.0332x over previous
"""Distributed masked-attention kernel for 8 TRN2 NeuronCores (v3).

Reference computation (B=2, L=1024, D=1024, H=16, DH=64):
    Qz, Kz = masked Q, K;  Qp/Kp/Vp = projections (V = K)
    per-head attention with outer-product validity mask, softmax scaled
    by 1/sqrt(D);  O = Qp + attn;  out = O + relu(mask_q(O @ Wo.T))

Sharding: core c = 2*g + b handles batch b = c%2, head group g = c//2
(4 heads, feature block e = [256g, 256g+256)).  Activations are
feature-major ("X.T" = [features, tokens]) so the TensorEngine contracts
along partitions without transposes; the host pre-transposes Q, K and
the weight shards and casts to bf16 (PSUM accumulation stays f32).

Softmax: scores are small (|S/32| < ~2) so no max-subtraction; exp(S/32 +
bias_k) with bias_k = -30000 at masked k gives exact zeros; the
denominator comes from a ones-column appended to V (M=65 matmul) plus a
1e30 PSUM prefill at masked-q columns so 1/denom ~ 0 there.

Attention runs q-chunk-major (512 tokens per chunk, head pairs packed
into PE row groups, exp pair-fused over 2 PSUM banks).  After each
q-chunk an 8-core AllToAll (64-token chunks per receiver, 0.5MB)
redistributes O.T; the first collective overlaps the second q-chunk.
Each core ends up with 64-token blocks of both batches and both q-halves
for the output-projection epilogue (two passes, one per collective).
The host reassembles the final [2,1024,1024].
"""
import numpy as np

B, L, D = 2, 1024, 1024
H, DH = 16, 64
NCORES = 8
HPC = 4          # heads per core
EB = 256         # feature block per core
NEG = -30000.0   # masked-k bias (exp -> exact 0)
BIG = 1e30       # masked-q denominator prefill

TRACE = False
TRACE_KWARGS = {}
LAST_RESULTS = None

_compiled = None


def _build():
    import concourse.bacc as bacc
    import concourse.tile as tile
    from concourse.tile import add_dep_helper
    from concourse import mybir

    f32 = mybir.dt.float32
    bf16 = mybir.dt.bfloat16
    EXP = mybir.ActivationFunctionType.Exp
    RELU = mybir.ActivationFunctionType.Relu

    nc = bacc.Bacc("TRN2", target_bir_lowering=False, debug=False,
                   num_devices=NCORES)

    qt = nc.dram_tensor("qt", [D, L], bf16, kind="ExternalInput")
    kt = nc.dram_tensor("kt", [D, L], bf16, kind="ExternalInput")
    wq = nc.dram_tensor("wq", [D, EB], bf16, kind="ExternalInput")
    wk = nc.dram_tensor("wk", [D, EB], bf16, kind="ExternalInput")
    wv = nc.dram_tensor("wv", [D, EB], bf16, kind="ExternalInput")
    wo = nc.dram_tensor("wo", [D, D], bf16, kind="ExternalInput")
    bk = nc.dram_tensor("bk", [128, 8], f32, kind="ExternalInput")
    e64 = nc.dram_tensor("e64", [1, 65], bf16, kind="ExternalInput")
    mvn = nc.dram_tensor("mvn", [1, L], bf16, kind="ExternalInput")
    mv = nc.dram_tensor("mv", [1, L], f32, kind="ExternalInput")
    out = nc.dram_tensor("out", [D, 256], f32, kind="ExternalOutput")

    with tile.TileContext(nc) as tc:
        with (
            tc.tile_pool(name="sb", bufs=1) as sb,
            tc.tile_pool(name="rot", bufs=3) as rot,
            tc.tile_pool(name="ps_big", bufs=2, space="PSUM") as ps_big,
            tc.tile_pool(name="ps_at", bufs=2, space="PSUM") as ps_at,
            tc.tile_pool(name="ps_sm", bufs=2, space="PSUM") as ps_sm,
            tc.tile_pool(name="dram", bufs=1, space="DRAM") as dram,
        ):
            # ---- constants / masks (tiny, first) ----
            bk_t = sb.tile([128, 8], f32, tag="bk")
            e64_t = sb.tile([1, 65], bf16, tag="e64")
            mvn_t = sb.tile([1, L], bf16, tag="mvn")
            mv_t = sb.tile([1, L], f32, tag="mv")
            nc.sync.dma_start(bk_t[:], bk[:])
            nc.sync.dma_start(e64_t[:], e64[:])
            nc.sync.dma_start(mvn_t[:], mvn[:])
            nc.sync.dma_start(mv_t[:], mv[:])
            mvbc = sb.tile([128, L], f32, tag="mvbc")
            nc.gpsimd.partition_broadcast(mvbc[:], mv_t[:])

            # ---- input DMAs, interleaved so dc-major compute starts early ----
            qt_t = [sb.tile([128, L], bf16, tag=f"qt{i}", name=f"qt{i}")
                    for i in range(8)]
            kt_t = [sb.tile([128, L], bf16, tag=f"kt{i}", name=f"kt{i}")
                    for i in range(8)]
            wq_t = [sb.tile([128, EB], bf16, tag=f"wq{i}", name=f"wq{i}")
                    for i in range(8)]
            wk_t = [sb.tile([128, EB], bf16, tag=f"wk{i}", name=f"wk{i}")
                    for i in range(8)]
            wv_t = [sb.tile([128, EB], bf16, tag=f"wv{i}", name=f"wv{i}")
                    for i in range(8)]
            for i in range(8):
                nc.sync.dma_start(wq_t[i][:], wq[128 * i:128 * (i + 1), :])
                nc.sync.dma_start(qt_t[i][:], qt[128 * i:128 * (i + 1), :])
            for i in range(8):
                nc.sync.dma_start(wk_t[i][:], wk[128 * i:128 * (i + 1), :])
                nc.sync.dma_start(kt_t[i][:], kt[128 * i:128 * (i + 1), :])
                nc.sync.dma_start(wv_t[i][:], wv[128 * i:128 * (i + 1), :])

            # ---- HAM warmup: dummy matmuls while DMAs land ----
            warm_ps = ps_at.tile([65, 512], f32, tag="at", name="warm_ps")
            for w in range(8):
                nc.tensor.matmul(warm_ps[:], e64_t[:],
                                 mvn_t[:, 0:512], start=(w == 0), stop=(w == 7))

            # ---- phase 1: projections, dc-major so PE starts on first chunk --
            qpt = [sb.tile([128, L], bf16, tag=f"qpt{i}", name=f"qpt{i}")
                   for i in range(2)]
            kpt = [sb.tile([128, L], bf16, tag=f"kpt{i}", name=f"kpt{i}")
                   for i in range(2)]
            for dst, w_t, x_t in ((qpt, wq_t, qt_t), (kpt, wk_t, kt_t)):
                pj = [ps_big.tile([128, L], f32, tag="big", name=f"pj{id(dst)}{et}")
                      for et in range(2)]
                for dc in range(8):
                    for et in range(2):
                        for qc in range(2):
                            nc.tensor.matmul(
                                pj[et][:, 512 * qc:512 * (qc + 1)],
                                w_t[dc][:, 128 * et:128 * (et + 1)],
                                x_t[dc][:, 512 * qc:512 * (qc + 1)],
                                start=(dc == 0), stop=(dc == 7))
                for et in range(2):
                    if dst is qpt:
                        # fold query-mask into Qp (residual uses masked Qp)
                        nc.vector.tensor_mul(dst[et][:], pj[et][:], mvbc[:])
                    else:
                        nc.vector.tensor_copy(dst[et][:], pj[et][:])

            # Vp natural [k-tokens, e] with ones column per head (65-stride)
            vpa = [sb.tile([128, 65 * HPC], bf16, tag=f"vpa{i}", name=f"vpa{i}")
                   for i in range(8)]
            for tt in range(8):
                nc.gpsimd.memset(vpa[tt][:], 1.0)
                pv = ps_sm.tile([128, EB], f32, tag="sm", name=f"pv{tt}")
                for dc in range(8):
                    nc.tensor.matmul(
                        pv[:], kt_t[dc][:, 128 * tt:128 * (tt + 1)], wv_t[dc][:],
                        start=(dc == 0), stop=(dc == 7))
                for h in range(HPC):
                    nc.vector.tensor_copy(
                        vpa[tt][:, 65 * h:65 * h + 64],
                        pv[:, 64 * h:64 * (h + 1)])

            # ---- epilogue weights (prefetch, after phase-1 loads) ----
            wo_t = [sb.tile([128, D], bf16, tag=f"wo{i}", name=f"wo{i}")
                    for i in range(8)]
            for i in range(8):
                nc.sync.dma_start(wo_t[i][:], wo[128 * i:128 * (i + 1), :])

            # ---- phase 2+3: attention q-chunk-major; A2A per q-chunk ----
            # A2A chunk j (-> rank j): [256 e, 64 t] at tokens 512*qc + 64*j.
            inb = [dram.tile([2048, 64], bf16, tag=f"inb{i}", name=f"inb{i}")
                   for i in range(2)]
            outb = [dram.tile([2048, 64], bf16, tag=f"outb{i}", name=f"outb{i}")
                    for i in range(2)]
            attn = [sb.tile([128, L], bf16, tag=f"attn{i}", name=f"attn{i}")
                    for i in range(2)]
            ot = [sb.tile([128, L], bf16, tag=f"ot{i}", name=f"ot{i}")
                  for i in range(2)]
            # ot_sl[dt] columns: [qc0-b0 | qc0-b1 | qc1-b0 | qc1-b1], 64 each
            ot_sl = [sb.tile([128, 256], bf16, tag=f"osl{i}", name=f"osl{i}")
                     for i in range(8)]
            from concourse import mybir as _mb

            last_attn_mm = None
            for qc in range(2):
                qs = slice(512 * qc, 512 * (qc + 1))
                for hp in (0, 2):
                    et = hp // 2
                    ats = []
                    for h in (hp, hp + 1):
                        at = ps_at.tile([65, 512], f32, tag="at",
                                        name=f"at{h}_{qc}")
                        nc.tensor.matmul(at[:], e64_t[:], mvn_t[:, qs],
                                         start=True, stop=False)
                        ats.append(at)
                    # software pipeline: S(k) issued ahead of attn(k-1)
                    p_prev = None
                    for ki in range(8):
                        ks = slice(128 * ki, 128 * (ki + 1))
                        s_ps = ps_big.tile([128, 1024], f32, tag="big",
                                           name=f"s{hp}_{qc}_{ki}")
                        for j in range(2):
                            ro = 64 * j
                            nc.tensor.matmul(
                                s_ps[:, 512 * j:512 * (j + 1)],
                                kpt[et][ro:ro + 64, ks],
                                qpt[et][ro:ro + 64, qs],
                                start=True, stop=True)
                        p_t = rot.tile([128, 1024], bf16, tag="p",
                                       name=f"p{hp}_{qc}_{ki}")
                        nc.scalar.activation(p_t[:], s_ps[:], EXP,
                                             bias=bk_t[:, ki:ki + 1],
                                             scale=1.0 / 32)
                        if p_prev is not None:
                            kp_, pp = p_prev
                            for j, h in enumerate((hp, hp + 1)):
                                nc.tensor.matmul(
                                    ats[j][:], vpa[kp_][:, 65 * h:65 * h + 65],
                                    pp[:, 512 * j:512 * (j + 1)],
                                    start=False, stop=False)
                        p_prev = (ki, p_t)
                    kp_, pp = p_prev
                    for j, h in enumerate((hp, hp + 1)):
                        last_attn_mm = nc.tensor.matmul(
                            ats[j][:], vpa[kp_][:, 65 * h:65 * h + 65],
                            pp[:, 512 * j:512 * (j + 1)],
                            start=False, stop=True)
                    # normalize: attn = at[0:64] / denom  (denom row 64)
                    for j, h in enumerate((hp, hp + 1)):
                        ro = 64 * (h % 2)
                        den = rot.tile([1, 512], f32, tag="den")
                        nc.vector.tensor_copy(den[:], ats[j][64:65, :])
                        rcp = rot.tile([1, 512], f32, tag="rcp")
                        nc.vector.reciprocal_approx_fast(rcp[:], den[:])
                        bc = rot.tile([64, 512], f32, tag="bc")
                        nc.gpsimd.partition_broadcast(bc[:], rcp[:])
                        nc.vector.tensor_mul(
                            attn[et][ro:ro + 64, qs], ats[j][0:64, :], bc[:])
                # residual for this q-chunk, then A2A redistribute
                for et in range(2):
                    nc.vector.tensor_add(ot[et][:, qs], qpt[et][:, qs],
                                         attn[et][:, qs])
                for et in range(2):
                    # one DMA: [p, j, c] -> inb rows 256j + 128et + p, col c
                    src = ot[et][:, qs].rearrange("p (j c) -> p j c", j=8)
                    dstv = inb[qc][:].rearrange("(j h p) c -> j h p c",
                                                j=8, h=2)[:, et, :, :]
                    dst = dstv.rearrange("j p c -> p j c")
                    nc.sync.dma_start(dst, src)
                nc.gpsimd.collective_compute(
                    "AllToAll", _mb.AluOpType.bypass,
                    replica_groups=[list(range(NCORES))],
                    ins=[inb[qc][:].opt()], outs=[outb[qc][:].opt()])

            # read back (after both triggers so SP FIFO never blocks a trigger)
            for qc in range(2):
                for dt_ in range(8):
                    g, hh = dt_ // 2, dt_ % 2
                    # rows (g b hh p): chunk j = 2g + b (core id = 2g + b)
                    src = outb[qc][:].rearrange(
                        "(g b h p) c -> g b h p c", g=4, b=2, h=2)[g, :, hh, :, :]
                    src = src.rearrange("b p c -> p b c")
                    dst = ot_sl[dt_][:, 128 * qc:128 * (qc + 1)].rearrange(
                        "p (b c) -> p b c", b=2)
                    nc.sync.dma_start(dst, src)

            # ---- phase 4: output projection epilogue (pass per q-chunk) ----
            prev_pass_mm = last_attn_mm
            for p in range(2):
                first_mms = []
                pass_last = None
                for et in range(8):
                    fpt = ps_sm.tile([128, 128], f32, tag="sm", name=f"fp{p}_{et}")
                    for dc in range(8):
                        mm = nc.tensor.matmul(
                            fpt[:], wo_t[dc][:, 128 * et:128 * (et + 1)],
                            ot_sl[dc][:, 128 * p:128 * (p + 1)],
                            start=(dc == 0), stop=(dc == 7))
                        if dc == 0:
                            first_mms.append(mm)
                        pass_last = mm
                    ff = rot.tile([128, 128], f32, tag="ff")
                    nc.scalar.activation(ff[:], fpt[:], RELU)
                    o_t = rot.tile([128, 128], f32, tag="outt")
                    nc.vector.tensor_add(o_t[:], ff[:],
                                         ot_sl[et][:, 128 * p:128 * (p + 1)])
                    nc.sync.dma_start(
                        out[128 * et:128 * (et + 1), 128 * p:128 * (p + 1)],
                        o_t[:])
                # pin PE order: this pass's first MMs after previous pass/attn
                for mm in first_mms:
                    add_dep_helper(mm.ins, prev_pass_mm.ins, sync=False,
                                   reason="epilogue pass ordering")
                prev_pass_mm = pass_last

    nc.compile()
    return nc


def _get_compiled():
    global _compiled
    if _compiled is None:
        _compiled = _build()
    return _compiled


def kernel(Q, K, mask_Q, mask_K, Wq, Wk, Wv, Wo):
    global LAST_RESULTS
    import ml_dtypes
    from concourse.bass_utils import run_bass_kernel_spmd

    bf = ml_dtypes.bfloat16
    Q = np.asarray(Q, np.float32)
    K = np.asarray(K, np.float32)
    mask_Q = np.asarray(mask_Q, bool)
    mask_K = np.asarray(mask_K, bool)
    Wq = np.asarray(Wq, np.float32)
    Wk = np.asarray(Wk, np.float32)
    Wv = np.asarray(Wv, np.float32)
    Wo = np.asarray(Wo, np.float32)

    nc = _get_compiled()

    e64v = np.zeros((1, 65), np.float32)
    e64v[0, 64] = BIG
    wot = np.ascontiguousarray(Wo.T.astype(bf))
    in_maps = []
    for c in range(NCORES):
        b, g = c % 2, c // 2
        eb = slice(EB * g, EB * (g + 1))
        bias = np.where(mask_K[b], NEG, 0.0).astype(np.float32)
        in_maps.append({
            "qt": np.ascontiguousarray(Q[b].T.astype(bf)),
            "kt": np.ascontiguousarray(K[b].T.astype(bf)),
            "wq": np.ascontiguousarray(Wq[eb, :].T.astype(bf)),
            "wk": np.ascontiguousarray(Wk[eb, :].T.astype(bf)),
            "wv": np.ascontiguousarray(Wv[eb, :].T.astype(bf)),
            "wo": wot,
            "bk": np.ascontiguousarray(bias.reshape(8, 128).T),
            "e64": e64v.astype(bf),
            "mvn": mask_Q[b].astype(bf)[None, :],
            "mv": (~mask_Q[b]).astype(np.float32)[None, :],
        })

    res = run_bass_kernel_spmd(nc, in_maps, core_ids=list(range(NCORES)),
                               trace=TRACE, **TRACE_KWARGS)
    LAST_RESULTS = res

    full = np.empty((B, L, D), np.float32)
    for c in range(NCORES):
        o = res.results[c]["out"]   # [1024 e, 256]: [qc0-b0|qc0-b1|qc1-b0|qc1-b1]
        full[0, 64 * c:64 * (c + 1), :] = o[:, 0:64].T
        full[1, 64 * c:64 * (c + 1), :] = o[:, 64:128].T
        full[0, 512 + 64 * c:512 + 64 * (c + 1), :] = o[:, 128:192].T
        full[1, 512 + 64 * c:512 + 64 * (c + 1), :] = o[:, 192:256].T
    return full


# revision 7
# speedup vs baseline: 1.3218x; 1.0078x over previous
"""Distributed masked-attention kernel for 8 TRN2 NeuronCores (v3).

Reference computation (B=2, L=1024, D=1024, H=16, DH=64):
    Qz, Kz = masked Q, K;  Qp/Kp/Vp = projections (V = K)
    per-head attention with outer-product validity mask, softmax scaled
    by 1/sqrt(D);  O = Qp + attn;  out = O + relu(mask_q(O @ Wo.T))

Sharding: core c = 2*g + b handles batch b = c%2, head group g = c//2
(4 heads, feature block e = [256g, 256g+256)).  Activations are
feature-major ("X.T" = [features, tokens]) so the TensorEngine contracts
along partitions without transposes; the host pre-transposes Q, K and
the weight shards and casts to bf16 (PSUM accumulation stays f32).

Softmax: scores are small (|S/32| < ~2) so no max-subtraction; exp(S/32 +
bias_k) with bias_k = -30000 at masked k gives exact zeros; the
denominator comes from a ones-column appended to V (M=65 matmul) plus a
1e30 PSUM prefill at masked-q columns so 1/denom ~ 0 there.

Attention runs q-chunk-major (512 tokens per chunk, head pairs packed
into PE row groups, exp pair-fused over 2 PSUM banks).  After each
q-chunk an 8-core AllToAll (64-token chunks per receiver, 0.5MB)
redistributes O.T; the first collective overlaps the second q-chunk.
Each core ends up with 64-token blocks of both batches and both q-halves
for the output-projection epilogue (two passes, one per collective).
The host reassembles the final [2,1024,1024].
"""
import numpy as np

B, L, D = 2, 1024, 1024
H, DH = 16, 64
NCORES = 8
HPC = 4          # heads per core
EB = 256         # feature block per core
NEG = -30000.0   # masked-k bias (exp -> exact 0)
BIG = 1e30       # masked-q denominator prefill

TRACE = False
TRACE_KWARGS = {}
LAST_RESULTS = None

_compiled = None


def _build():
    import concourse.bacc as bacc
    import concourse.tile as tile
    from concourse.tile import add_dep_helper
    from concourse import mybir

    f32 = mybir.dt.float32
    bf16 = mybir.dt.bfloat16
    EXP = mybir.ActivationFunctionType.Exp
    RELU = mybir.ActivationFunctionType.Relu

    nc = bacc.Bacc("TRN2", target_bir_lowering=False, debug=False,
                   num_devices=NCORES)

    qt = nc.dram_tensor("qt", [D, L], bf16, kind="ExternalInput")
    kt = nc.dram_tensor("kt", [D, L], bf16, kind="ExternalInput")
    wq = nc.dram_tensor("wq", [D, EB], bf16, kind="ExternalInput")
    wk = nc.dram_tensor("wk", [D, EB], bf16, kind="ExternalInput")
    wv = nc.dram_tensor("wv", [D, EB], bf16, kind="ExternalInput")
    wo = nc.dram_tensor("wo", [D, D], bf16, kind="ExternalInput")
    bk = nc.dram_tensor("bk", [128, 8], f32, kind="ExternalInput")
    e64 = nc.dram_tensor("e64", [1, 65], bf16, kind="ExternalInput")
    mvn = nc.dram_tensor("mvn", [1, L], bf16, kind="ExternalInput")
    mv = nc.dram_tensor("mv", [1, L], f32, kind="ExternalInput")
    out = nc.dram_tensor("out", [D, 256], f32, kind="ExternalOutput")

    with tile.TileContext(nc) as tc:
        with (
            tc.tile_pool(name="sb", bufs=1) as sb,
            tc.tile_pool(name="rot", bufs=3) as rot,
            tc.tile_pool(name="ps_big", bufs=2, space="PSUM") as ps_big,
            tc.tile_pool(name="ps_at", bufs=4, space="PSUM") as ps_at,
            tc.tile_pool(name="dram", bufs=1, space="DRAM") as dram,
        ):
            # ---- constants / masks (tiny, first) ----
            bk_t = sb.tile([128, 8], f32, tag="bk")
            e64_t = sb.tile([1, 65], bf16, tag="e64")
            mvn_t = sb.tile([1, L], bf16, tag="mvn")
            mv_t = sb.tile([1, L], f32, tag="mv")
            nc.sync.dma_start(bk_t[:], bk[:])
            nc.sync.dma_start(e64_t[:], e64[:])
            nc.sync.dma_start(mvn_t[:], mvn[:])
            nc.sync.dma_start(mv_t[:], mv[:])
            mvbc = sb.tile([128, L], f32, tag="mvbc")
            nc.gpsimd.partition_broadcast(mvbc[:], mv_t[:])

            # ---- consolidated input DMAs (few big strided loads) ----
            # Layout: big [128, n*F] tiles; chunk dc at cols [F*dc : F*(dc+1)]
            qt_b = sb.tile([128, 8 * L], bf16, tag="qtb")
            kt_b = sb.tile([128, 8 * L], bf16, tag="ktb")
            wq_b = sb.tile([128, 8 * EB], bf16, tag="wqb")
            wk_b = sb.tile([128, 8 * EB], bf16, tag="wkb")
            wv_b = sb.tile([128, 8 * EB], bf16, tag="wvb")

            def load_chunked(eng, dst_big, src_dram, F, lo, hi):
                n = hi - lo
                dst = dst_big[:, F * lo:F * hi].rearrange(
                    "p (dc t) -> p dc t", dc=n)
                srcv = src_dram[128 * lo:128 * hi, :].rearrange(
                    "(dc p) t -> p dc t", p=128)
                eng.dma_start(dst, srcv)

            load_chunked(nc.sync, wq_b, wq, EB, 0, 8)
            load_chunked(nc.sync, qt_b, qt, L, 0, 4)
            load_chunked(nc.sync, qt_b, qt, L, 4, 8)
            load_chunked(nc.scalar, wk_b, wk, EB, 0, 8)
            load_chunked(nc.scalar, kt_b, kt, L, 0, 4)
            load_chunked(nc.scalar, kt_b, kt, L, 4, 8)
            load_chunked(nc.scalar, wv_b, wv, EB, 0, 8)
            qt_t = [qt_b[:, L * i:L * (i + 1)] for i in range(8)]
            kt_t = [kt_b[:, L * i:L * (i + 1)] for i in range(8)]
            wq_t = [wq_b[:, EB * i:EB * (i + 1)] for i in range(8)]
            wk_t = [wk_b[:, EB * i:EB * (i + 1)] for i in range(8)]
            wv_t = [wv_b[:, EB * i:EB * (i + 1)] for i in range(8)]

            # ---- HAM warmup: dummy matmuls while DMAs land ----
            warm_ps = ps_at.tile([65, 512], f32, tag="at", name="warm_ps")
            for w in range(8):
                nc.tensor.matmul(warm_ps[:], e64_t[:],
                                 mvn_t[:, 0:512], start=(w == 0), stop=(w == 7))

            # ---- phase 1: projections, dc-major so PE starts on first chunk --
            qpt = [sb.tile([128, L], bf16, tag=f"qpt{i}", name=f"qpt{i}")
                   for i in range(2)]
            kpt = [sb.tile([128, L], bf16, tag=f"kpt{i}", name=f"kpt{i}")
                   for i in range(2)]
            for dst, w_t, x_t in ((qpt, wq_t, qt_t), (kpt, wk_t, kt_t)):
                pj = [ps_big.tile([128, L], f32, tag="big", name=f"pj{id(dst)}{et}")
                      for et in range(2)]
                for dc in range(8):
                    for et in range(2):
                        for qc in range(2):
                            nc.tensor.matmul(
                                pj[et][:, 512 * qc:512 * (qc + 1)],
                                w_t[dc][:, 128 * et:128 * (et + 1)],
                                x_t[dc][:, 512 * qc:512 * (qc + 1)],
                                start=(dc == 0), stop=(dc == 7))
                for et in range(2):
                    if dst is qpt:
                        # fold query-mask into Qp (residual uses masked Qp)
                        nc.vector.tensor_mul(dst[et][:], pj[et][:], mvbc[:])
                    else:
                        nc.vector.tensor_copy(dst[et][:], pj[et][:])

            # Vp natural [k-tokens, e] with ones column per head (65-stride)
            vpa = [sb.tile([128, 65 * HPC], bf16, tag=f"vpa{i}", name=f"vpa{i}")
                   for i in range(8)]
            for tt in range(8):
                nc.gpsimd.memset(vpa[tt][:], 1.0)
                pv = ps_big.tile([128, EB], f32, tag="big", name=f"pv{tt}")
                for dc in range(8):
                    nc.tensor.matmul(
                        pv[:], kt_t[dc][:, 128 * tt:128 * (tt + 1)], wv_t[dc][:],
                        start=(dc == 0), stop=(dc == 7))
                for h in range(HPC):
                    nc.vector.tensor_copy(
                        vpa[tt][:, 65 * h:65 * h + 64],
                        pv[:, 64 * h:64 * (h + 1)])

            # ---- epilogue weights (prefetch, after phase-1 loads) ----
            wo_b = sb.tile([128, 8 * D], bf16, tag="wob")
            load_chunked(nc.scalar, wo_b, wo, D, 0, 4)
            load_chunked(nc.scalar, wo_b, wo, D, 4, 8)
            wo_t = [wo_b[:, D * i:D * (i + 1)] for i in range(8)]

            # ---- phase 2+3: attention q-chunk-major; A2A per q-chunk ----
            # A2A chunk j (-> rank j): [256 e, 64 t] at tokens 512*qc + 64*j.
            inb = [dram.tile([2048, 64], bf16, tag=f"inb{i}", name=f"inb{i}")
                   for i in range(2)]
            outb = [dram.tile([2048, 64], bf16, tag=f"outb{i}", name=f"outb{i}")
                    for i in range(2)]
            attn = [sb.tile([128, L], bf16, tag=f"attn{i}", name=f"attn{i}")
                    for i in range(2)]
            ot = [sb.tile([128, L], bf16, tag=f"ot{i}", name=f"ot{i}")
                  for i in range(2)]
            # ot_sl[dt] columns: [qc0-b0 | qc0-b1 | qc1-b0 | qc1-b1], 64 each
            ot_sl = [sb.tile([128, 256], bf16, tag=f"osl{i}", name=f"osl{i}")
                     for i in range(8)]
            from concourse import mybir as _mb

            last_attn_mm = None
            for qc in range(2):
                qs = slice(512 * qc, 512 * (qc + 1))
                for hp in (0, 2):
                    et = hp // 2
                    ats = []
                    for h in (hp, hp + 1):
                        at = ps_at.tile([65, 512], f32, tag="at",
                                        name=f"at{h}_{qc}")
                        nc.tensor.matmul(at[:], e64_t[:], mvn_t[:, qs],
                                         start=True, stop=False)
                        ats.append(at)
                    # software pipeline: S(k) issued ahead of attn(k-1)
                    p_prev = None
                    for ki in range(8):
                        ks = slice(128 * ki, 128 * (ki + 1))
                        s_ps = ps_big.tile([128, 1024], f32, tag="big",
                                           name=f"s{hp}_{qc}_{ki}")
                        for j in range(2):
                            ro = 64 * j
                            nc.tensor.matmul(
                                s_ps[:, 512 * j:512 * (j + 1)],
                                kpt[et][ro:ro + 64, ks],
                                qpt[et][ro:ro + 64, qs],
                                start=True, stop=True)
                        p_t = rot.tile([128, 1024], bf16, tag="p",
                                       name=f"p{hp}_{qc}_{ki}")
                        nc.scalar.activation(p_t[:], s_ps[:], EXP,
                                             bias=bk_t[:, ki:ki + 1],
                                             scale=1.0 / 32)
                        if p_prev is not None:
                            kp_, pp = p_prev
                            for j, h in enumerate((hp, hp + 1)):
                                nc.tensor.matmul(
                                    ats[j][:], vpa[kp_][:, 65 * h:65 * h + 65],
                                    pp[:, 512 * j:512 * (j + 1)],
                                    start=False, stop=False)
                        p_prev = (ki, p_t)
                    kp_, pp = p_prev
                    for j, h in enumerate((hp, hp + 1)):
                        last_attn_mm = nc.tensor.matmul(
                            ats[j][:], vpa[kp_][:, 65 * h:65 * h + 65],
                            pp[:, 512 * j:512 * (j + 1)],
                            start=False, stop=True)
                    # normalize: attn = at[0:64] / denom  (denom row 64)
                    for j, h in enumerate((hp, hp + 1)):
                        ro = 64 * (h % 2)
                        den = rot.tile([1, 512], f32, tag="den")
                        nc.vector.tensor_copy(den[:], ats[j][64:65, :])
                        rcp = rot.tile([1, 512], f32, tag="rcp")
                        nc.vector.reciprocal_approx_fast(rcp[:], den[:])
                        bc = rot.tile([64, 512], f32, tag="bc")
                        nc.gpsimd.partition_broadcast(bc[:], rcp[:])
                        nc.vector.tensor_mul(
                            attn[et][ro:ro + 64, qs], ats[j][0:64, :], bc[:])
                # residual for this q-chunk, then A2A redistribute
                for et in range(2):
                    nc.vector.tensor_add(ot[et][:, qs], qpt[et][:, qs],
                                         attn[et][:, qs])
                for et in range(2):
                    # one DMA: [p, j, c] -> inb rows 256j + 128et + p, col c
                    src = ot[et][:, qs].rearrange("p (j c) -> p j c", j=8)
                    dstv = inb[qc][:].rearrange("(j h p) c -> j h p c",
                                                j=8, h=2)[:, et, :, :]
                    dst = dstv.rearrange("j p c -> p j c")
                    nc.sync.dma_start(dst, src)
                nc.gpsimd.collective_compute(
                    "AllToAll", _mb.AluOpType.bypass,
                    replica_groups=[list(range(NCORES))],
                    ins=[inb[qc][:].opt()], outs=[outb[qc][:].opt()])

            # read back (after both triggers so SP FIFO never blocks a trigger)
            for qc in range(2):
                for dt_ in range(8):
                    g, hh = dt_ // 2, dt_ % 2
                    # rows (g b hh p): chunk j = 2g + b (core id = 2g + b)
                    src = outb[qc][:].rearrange(
                        "(g b h p) c -> g b h p c", g=4, b=2, h=2)[g, :, hh, :, :]
                    src = src.rearrange("b p c -> p b c")
                    dst = ot_sl[dt_][:, 128 * qc:128 * (qc + 1)].rearrange(
                        "p (b c) -> p b c", b=2)
                    nc.sync.dma_start(dst, src)

            # ---- phase 4: output projection epilogue (pass per q-chunk) ----
            prev_pass_mm = last_attn_mm
            for p in range(2):
                first_mms = []
                pass_last = None
                for et in range(8):
                    fpt = ps_big.tile([128, 128], f32, tag="big", name=f"fp{p}_{et}")
                    for dc in range(8):
                        mm = nc.tensor.matmul(
                            fpt[:], wo_t[dc][:, 128 * et:128 * (et + 1)],
                            ot_sl[dc][:, 128 * p:128 * (p + 1)],
                            start=(dc == 0), stop=(dc == 7))
                        if dc == 0:
                            first_mms.append(mm)
                        pass_last = mm
                    ff = rot.tile([128, 128], f32, tag="ff")
                    nc.scalar.activation(ff[:], fpt[:], RELU)
                    o_t = rot.tile([128, 128], f32, tag="outt")
                    nc.vector.tensor_add(o_t[:], ff[:],
                                         ot_sl[et][:, 128 * p:128 * (p + 1)])
                    nc.sync.dma_start(
                        out[128 * et:128 * (et + 1), 128 * p:128 * (p + 1)],
                        o_t[:])
                # pin PE order: this pass's first MMs after previous pass/attn
                for mm in first_mms:
                    add_dep_helper(mm.ins, prev_pass_mm.ins, sync=False,
                                   reason="epilogue pass ordering")
                prev_pass_mm = pass_last

    nc.compile()
    return nc


def _get_compiled():
    global _compiled
    if _compiled is None:
        _compiled = _build()
    return _compiled


def kernel(Q, K, mask_Q, mask_K, Wq, Wk, Wv, Wo):
    global LAST_RESULTS
    import ml_dtypes
    from concourse.bass_utils import run_bass_kernel_spmd

    bf = ml_dtypes.bfloat16
    Q = np.asarray(Q, np.float32)
    K = np.asarray(K, np.float32)
    mask_Q = np.asarray(mask_Q, bool)
    mask_K = np.asarray(mask_K, bool)
    Wq = np.asarray(Wq, np.float32)
    Wk = np.asarray(Wk, np.float32)
    Wv = np.asarray(Wv, np.float32)
    Wo = np.asarray(Wo, np.float32)

    nc = _get_compiled()

    e64v = np.zeros((1, 65), np.float32)
    e64v[0, 64] = BIG
    wot = np.ascontiguousarray(Wo.T.astype(bf))
    in_maps = []
    for c in range(NCORES):
        b, g = c % 2, c // 2
        eb = slice(EB * g, EB * (g + 1))
        bias = np.where(mask_K[b], NEG, 0.0).astype(np.float32)
        in_maps.append({
            "qt": np.ascontiguousarray(Q[b].T.astype(bf)),
            "kt": np.ascontiguousarray(K[b].T.astype(bf)),
            "wq": np.ascontiguousarray(Wq[eb, :].T.astype(bf)),
            "wk": np.ascontiguousarray(Wk[eb, :].T.astype(bf)),
            "wv": np.ascontiguousarray(Wv[eb, :].T.astype(bf)),
            "wo": wot,
            "bk": np.ascontiguousarray(bias.reshape(8, 128).T),
            "e64": e64v.astype(bf),
            "mvn": mask_Q[b].astype(bf)[None, :],
            "mv": (~mask_Q[b]).astype(np.float32)[None, :],
        })

    res = run_bass_kernel_spmd(nc, in_maps, core_ids=list(range(NCORES)),
                               trace=TRACE, **TRACE_KWARGS)
    LAST_RESULTS = res

    full = np.empty((B, L, D), np.float32)
    for c in range(NCORES):
        o = res.results[c]["out"]   # [1024 e, 256]: [qc0-b0|qc0-b1|qc1-b0|qc1-b1]
        full[0, 64 * c:64 * (c + 1), :] = o[:, 0:64].T
        full[1, 64 * c:64 * (c + 1), :] = o[:, 64:128].T
        full[0, 512 + 64 * c:512 + 64 * (c + 1), :] = o[:, 128:192].T
        full[1, 512 + 64 * c:512 + 64 * (c + 1), :] = o[:, 192:256].T
    return full


# revision 8
# speedup vs baseline: 1.5722x; 1.1894x over previous
"""Distributed masked-attention kernel for 8 TRN2 NeuronCores (v3).

Reference computation (B=2, L=1024, D=1024, H=16, DH=64):
    Qz, Kz = masked Q, K;  Qp/Kp/Vp = projections (V = K)
    per-head attention with outer-product validity mask, softmax scaled
    by 1/sqrt(D);  O = Qp + attn;  out = O + relu(mask_q(O @ Wo.T))

Sharding: core c = 2*g + b handles batch b = c%2, head group g = c//2
(4 heads, feature block e = [256g, 256g+256)).  Activations are
feature-major ("X.T" = [features, tokens]) so the TensorEngine contracts
along partitions without transposes; the host pre-transposes Q, K and
the weight shards and casts to bf16 (PSUM accumulation stays f32).

Softmax: scores are small (|S/32| < ~2) so no max-subtraction; exp(S/32 +
bias_k) with bias_k = -30000 at masked k gives exact zeros; the
denominator comes from a ones-column appended to V (M=65 matmul) plus a
1e30 PSUM prefill at masked-q columns so 1/denom ~ 0 there.

Attention runs q-chunk-major (512 tokens per chunk, head pairs packed
into PE row groups, exp pair-fused over 2 PSUM banks).  After each
q-chunk an 8-core AllToAll (64-token chunks per receiver, 0.5MB)
redistributes O.T; the first collective overlaps the second q-chunk.
Each core ends up with 64-token blocks of both batches and both q-halves
for the output-projection epilogue (two passes, one per collective).
The host reassembles the final [2,1024,1024].
"""
import numpy as np

B, L, D = 2, 1024, 1024
H, DH = 16, 64
NCORES = 8
HPC = 4          # heads per core
EB = 256         # feature block per core
NEG = -30000.0   # masked-k bias (exp -> exact 0)
BIG = 1e30       # masked-q denominator prefill

TRACE = False
TRACE_KWARGS = {}
LAST_RESULTS = None

_compiled = None


def _build():
    import concourse.bacc as bacc
    import concourse.tile as tile
    from concourse.tile import add_dep_helper
    from concourse import mybir

    f32 = mybir.dt.float32
    bf16 = mybir.dt.bfloat16
    EXP = mybir.ActivationFunctionType.Exp
    RELU = mybir.ActivationFunctionType.Relu

    nc = bacc.Bacc("TRN2", target_bir_lowering=False, debug=False,
                   num_devices=NCORES)

    qt = nc.dram_tensor("qt", [D, L], bf16, kind="ExternalInput")
    kt = nc.dram_tensor("kt", [D, L], bf16, kind="ExternalInput")
    wq = nc.dram_tensor("wq", [D, EB], bf16, kind="ExternalInput")
    wk = nc.dram_tensor("wk", [D, EB], bf16, kind="ExternalInput")
    wv = nc.dram_tensor("wv", [D, EB], bf16, kind="ExternalInput")
    wo = nc.dram_tensor("wo", [D, D], bf16, kind="ExternalInput")
    bk = nc.dram_tensor("bk", [128, 8], f32, kind="ExternalInput")
    e64 = nc.dram_tensor("e64", [1, 65], bf16, kind="ExternalInput")
    mvn = nc.dram_tensor("mvn", [1, L], bf16, kind="ExternalInput")
    mv = nc.dram_tensor("mv", [1, L], f32, kind="ExternalInput")
    out = nc.dram_tensor("out", [D, 256], f32, kind="ExternalOutput")

    with tile.TileContext(nc) as tc:
        with (
            tc.tile_pool(name="sb", bufs=1) as sb,
            tc.tile_pool(name="rot", bufs=3) as rot,
            tc.tile_pool(name="ps_big", bufs=2, space="PSUM") as ps_big,
            tc.tile_pool(name="ps_at", bufs=4, space="PSUM") as ps_at,
            tc.tile_pool(name="dram", bufs=1, space="DRAM") as dram,
        ):
            # ---- constants / masks (tiny, first) ----
            bk_t = sb.tile([128, 8], f32, tag="bk")
            e64_t = sb.tile([1, 65], bf16, tag="e64")
            mvn_t = sb.tile([1, L], bf16, tag="mvn")
            mv_t = sb.tile([1, L], f32, tag="mv")
            nc.sync.dma_start(bk_t[:], bk[:])
            nc.sync.dma_start(e64_t[:], e64[:])
            nc.sync.dma_start(mvn_t[:], mvn[:])
            nc.sync.dma_start(mv_t[:], mv[:])
            mvbc = sb.tile([128, L], f32, tag="mvbc")
            nc.gpsimd.partition_broadcast(mvbc[:], mv_t[:])

            # ---- input DMAs: contiguous chunks, issue split over SP+ACT ----
            qt_t = [sb.tile([128, L], bf16, tag=f"qt{i}", name=f"qt{i}")
                    for i in range(8)]
            kt_t = [sb.tile([128, L], bf16, tag=f"kt{i}", name=f"kt{i}")
                    for i in range(8)]
            wq_t = [sb.tile([128, EB], bf16, tag=f"wq{i}", name=f"wq{i}")
                    for i in range(8)]
            wk_t = [sb.tile([128, EB], bf16, tag=f"wk{i}", name=f"wk{i}")
                    for i in range(8)]
            wv_t = [sb.tile([128, EB], bf16, tag=f"wv{i}", name=f"wv{i}")
                    for i in range(8)]
            for i in range(8):
                nc.sync.dma_start(wq_t[i][:], wq[128 * i:128 * (i + 1), :])
                nc.sync.dma_start(qt_t[i][:], qt[128 * i:128 * (i + 1), :])
                nc.scalar.dma_start(wk_t[i][:], wk[128 * i:128 * (i + 1), :])
                nc.scalar.dma_start(kt_t[i][:], kt[128 * i:128 * (i + 1), :])
                nc.scalar.dma_start(wv_t[i][:], wv[128 * i:128 * (i + 1), :])

            # ---- HAM warmup: dummy matmuls while DMAs land ----
            warm_ps = ps_at.tile([65, 512], f32, tag="at", name="warm_ps")
            for w in range(8):
                nc.tensor.matmul(warm_ps[:], e64_t[:],
                                 mvn_t[:, 0:512], start=(w == 0), stop=(w == 7))

            # ---- phase 1: projections, dc-major so PE starts on first chunk --
            qpt = [sb.tile([128, L], bf16, tag=f"qpt{i}", name=f"qpt{i}")
                   for i in range(2)]
            kpt = [sb.tile([128, L], bf16, tag=f"kpt{i}", name=f"kpt{i}")
                   for i in range(2)]
            for dst, w_t, x_t in ((qpt, wq_t, qt_t), (kpt, wk_t, kt_t)):
                pj = [ps_big.tile([128, L], f32, tag="big", name=f"pj{id(dst)}{et}")
                      for et in range(2)]
                for dc in range(8):
                    for et in range(2):
                        for qc in range(2):
                            nc.tensor.matmul(
                                pj[et][:, 512 * qc:512 * (qc + 1)],
                                w_t[dc][:, 128 * et:128 * (et + 1)],
                                x_t[dc][:, 512 * qc:512 * (qc + 1)],
                                start=(dc == 0), stop=(dc == 7))
                for et in range(2):
                    if dst is qpt:
                        # fold query-mask into Qp (residual uses masked Qp)
                        nc.vector.tensor_mul(dst[et][:], pj[et][:], mvbc[:])
                    else:
                        nc.vector.tensor_copy(dst[et][:], pj[et][:])

            # Vp natural [k-tokens, e] with ones column per head (65-stride)
            vpa = [sb.tile([128, 65 * HPC], bf16, tag=f"vpa{i}", name=f"vpa{i}")
                   for i in range(8)]
            for tt in range(8):
                nc.gpsimd.memset(vpa[tt][:], 1.0)
                pv = ps_big.tile([128, EB], f32, tag="big", name=f"pv{tt}")
                for dc in range(8):
                    nc.tensor.matmul(
                        pv[:], kt_t[dc][:, 128 * tt:128 * (tt + 1)], wv_t[dc][:],
                        start=(dc == 0), stop=(dc == 7))
                for h in range(HPC):
                    nc.vector.tensor_copy(
                        vpa[tt][:, 65 * h:65 * h + 64],
                        pv[:, 64 * h:64 * (h + 1)])

            # ---- epilogue weights (prefetch, after phase-1 loads) ----
            wo_t = [sb.tile([128, D], bf16, tag=f"wo{i}", name=f"wo{i}")
                    for i in range(8)]
            for i in range(8):
                nc.scalar.dma_start(wo_t[i][:], wo[128 * i:128 * (i + 1), :])

            # ---- phase 2+3: attention q-chunk-major; A2A per q-chunk ----
            # A2A chunk j (-> rank j): [256 e, 64 t] at tokens 512*qc + 64*j.
            inb = [dram.tile([2048, 64], bf16, tag=f"inb{i}", name=f"inb{i}")
                   for i in range(2)]
            outb = [dram.tile([2048, 64], bf16, tag=f"outb{i}", name=f"outb{i}")
                    for i in range(2)]
            attn = [sb.tile([128, L], bf16, tag=f"attn{i}", name=f"attn{i}")
                    for i in range(2)]
            ot = [sb.tile([128, L], bf16, tag=f"ot{i}", name=f"ot{i}")
                  for i in range(2)]
            # ot_sl[dt] columns: [qc0-b0 | qc0-b1 | qc1-b0 | qc1-b1], 64 each
            ot_sl = [sb.tile([128, 256], bf16, tag=f"osl{i}", name=f"osl{i}")
                     for i in range(8)]
            from concourse import mybir as _mb

            last_attn_mm = None
            for qc in range(2):
                qs = slice(512 * qc, 512 * (qc + 1))
                for hp in (0, 2):
                    et = hp // 2
                    ats = []
                    for h in (hp, hp + 1):
                        at = ps_at.tile([65, 512], f32, tag="at",
                                        name=f"at{h}_{qc}")
                        nc.tensor.matmul(at[:], e64_t[:], mvn_t[:, qs],
                                         start=True, stop=False)
                        ats.append(at)
                    # software pipeline: S(k) issued ahead of attn(k-1)
                    p_prev = None
                    for ki in range(8):
                        ks = slice(128 * ki, 128 * (ki + 1))
                        s_ps = ps_big.tile([128, 1024], f32, tag="big",
                                           name=f"s{hp}_{qc}_{ki}")
                        for j in range(2):
                            ro = 64 * j
                            nc.tensor.matmul(
                                s_ps[:, 512 * j:512 * (j + 1)],
                                kpt[et][ro:ro + 64, ks],
                                qpt[et][ro:ro + 64, qs],
                                start=True, stop=True)
                        p_t = rot.tile([128, 1024], bf16, tag="p",
                                       name=f"p{hp}_{qc}_{ki}")
                        nc.scalar.activation(p_t[:], s_ps[:], EXP,
                                             bias=bk_t[:, ki:ki + 1],
                                             scale=1.0 / 32)
                        if p_prev is not None:
                            kp_, pp = p_prev
                            for j, h in enumerate((hp, hp + 1)):
                                nc.tensor.matmul(
                                    ats[j][:], vpa[kp_][:, 65 * h:65 * h + 65],
                                    pp[:, 512 * j:512 * (j + 1)],
                                    start=False, stop=False)
                        p_prev = (ki, p_t)
                    kp_, pp = p_prev
                    for j, h in enumerate((hp, hp + 1)):
                        last_attn_mm = nc.tensor.matmul(
                            ats[j][:], vpa[kp_][:, 65 * h:65 * h + 65],
                            pp[:, 512 * j:512 * (j + 1)],
                            start=False, stop=True)
                    # normalize: attn = at[0:64] / denom  (denom row 64)
                    for j, h in enumerate((hp, hp + 1)):
                        ro = 64 * (h % 2)
                        den = rot.tile([1, 512], f32, tag="den")
                        nc.vector.tensor_copy(den[:], ats[j][64:65, :])
                        rcp = rot.tile([1, 512], f32, tag="rcp")
                        nc.vector.reciprocal_approx_fast(rcp[:], den[:])
                        bc = rot.tile([64, 512], f32, tag="bc")
                        nc.gpsimd.partition_broadcast(bc[:], rcp[:])
                        nc.vector.tensor_mul(
                            attn[et][ro:ro + 64, qs], ats[j][0:64, :], bc[:])
                # residual for this q-chunk, then A2A redistribute
                for et in range(2):
                    nc.vector.tensor_add(ot[et][:, qs], qpt[et][:, qs],
                                         attn[et][:, qs])
                for et in range(2):
                    # one DMA: [p, j, c] -> inb rows 256j + 128et + p, col c
                    src = ot[et][:, qs].rearrange("p (j c) -> p j c", j=8)
                    dstv = inb[qc][:].rearrange("(j h p) c -> j h p c",
                                                j=8, h=2)[:, et, :, :]
                    dst = dstv.rearrange("j p c -> p j c")
                    nc.sync.dma_start(dst, src)
                nc.gpsimd.collective_compute(
                    "AllToAll", _mb.AluOpType.bypass,
                    replica_groups=[list(range(NCORES))],
                    ins=[inb[qc][:].opt()], outs=[outb[qc][:].opt()])

            # read back (after both triggers so SP FIFO never blocks a trigger)
            for qc in range(2):
                for dt_ in range(8):
                    g, hh = dt_ // 2, dt_ % 2
                    # rows (g b hh p): chunk j = 2g + b (core id = 2g + b)
                    src = outb[qc][:].rearrange(
                        "(g b h p) c -> g b h p c", g=4, b=2, h=2)[g, :, hh, :, :]
                    src = src.rearrange("b p c -> p b c")
                    dst = ot_sl[dt_][:, 128 * qc:128 * (qc + 1)].rearrange(
                        "p (b c) -> p b c", b=2)
                    nc.sync.dma_start(dst, src)

            # ---- phase 4: output projection epilogue (pass per q-chunk) ----
            prev_pass_mm = last_attn_mm
            for p in range(2):
                first_mms = []
                pass_last = None
                for et in range(8):
                    fpt = ps_big.tile([128, 128], f32, tag="big", name=f"fp{p}_{et}")
                    for dc in range(8):
                        mm = nc.tensor.matmul(
                            fpt[:], wo_t[dc][:, 128 * et:128 * (et + 1)],
                            ot_sl[dc][:, 128 * p:128 * (p + 1)],
                            start=(dc == 0), stop=(dc == 7))
                        if dc == 0:
                            first_mms.append(mm)
                        pass_last = mm
                    ff = rot.tile([128, 128], f32, tag="ff")
                    nc.scalar.activation(ff[:], fpt[:], RELU)
                    o_t = rot.tile([128, 128], f32, tag="outt")
                    nc.vector.tensor_add(o_t[:], ff[:],
                                         ot_sl[et][:, 128 * p:128 * (p + 1)])
                    nc.sync.dma_start(
                        out[128 * et:128 * (et + 1), 128 * p:128 * (p + 1)],
                        o_t[:])
                # pin PE order: this pass's first MMs after previous pass/attn
                for mm in first_mms:
                    add_dep_helper(mm.ins, prev_pass_mm.ins, sync=False,
                                   reason="epilogue pass ordering")
                prev_pass_mm = pass_last

    nc.compile()
    return nc


def _get_compiled():
    global _compiled
    if _compiled is None:
        _compiled = _build()
    return _compiled


def kernel(Q, K, mask_Q, mask_K, Wq, Wk, Wv, Wo):
    global LAST_RESULTS
    import ml_dtypes
    from concourse.bass_utils import run_bass_kernel_spmd

    bf = ml_dtypes.bfloat16
    Q = np.asarray(Q, np.float32)
    K = np.asarray(K, np.float32)
    mask_Q = np.asarray(mask_Q, bool)
    mask_K = np.asarray(mask_K, bool)
    Wq = np.asarray(Wq, np.float32)
    Wk = np.asarray(Wk, np.float32)
    Wv = np.asarray(Wv, np.float32)
    Wo = np.asarray(Wo, np.float32)

    nc = _get_compiled()

    e64v = np.zeros((1, 65), np.float32)
    e64v[0, 64] = BIG
    wot = np.ascontiguousarray(Wo.T.astype(bf))
    in_maps = []
    for c in range(NCORES):
        b, g = c % 2, c // 2
        eb = slice(EB * g, EB * (g + 1))
        bias = np.where(mask_K[b], NEG, 0.0).astype(np.float32)
        in_maps.append({
            "qt": np.ascontiguousarray(Q[b].T.astype(bf)),
            "kt": np.ascontiguousarray(K[b].T.astype(bf)),
            "wq": np.ascontiguousarray(Wq[eb, :].T.astype(bf)),
            "wk": np.ascontiguousarray(Wk[eb, :].T.astype(bf)),
            "wv": np.ascontiguousarray(Wv[eb, :].T.astype(bf)),
            "wo": wot,
            "bk": np.ascontiguousarray(bias.reshape(8, 128).T),
            "e64": e64v.astype(bf),
            "mvn": mask_Q[b].astype(bf)[None, :],
            "mv": (~mask_Q[b]).astype(np.float32)[None, :],
        })

    res = run_bass_kernel_spmd(nc, in_maps, core_ids=list(range(NCORES)),
                               trace=TRACE, **TRACE_KWARGS)
    LAST_RESULTS = res

    full = np.empty((B, L, D), np.float32)
    for c in range(NCORES):
        o = res.results[c]["out"]   # [1024 e, 256]: [qc0-b0|qc0-b1|qc1-b0|qc1-b1]
        full[0, 64 * c:64 * (c + 1), :] = o[:, 0:64].T
        full[1, 64 * c:64 * (c + 1), :] = o[:, 64:128].T
        full[0, 512 + 64 * c:512 + 64 * (c + 1), :] = o[:, 128:192].T
        full[1, 512 + 64 * c:512 + 64 * (c + 1), :] = o[:, 192:256].T
    return full
